# revision 1
# baseline (speedup 1.0000x reference)
"""Trainium2 Bass kernel for nn_CUBASpikingCNN (spiking CNN, T=100 steps).

Strategy: data-parallel over batch (B=32 -> 4 per core x 8 cores). Per core,
the network is processed layer-phase by layer-phase in t-chunks of 10:
  - conv psp for a whole chunk via batched matmuls (biases folded in via
    K=1 ones-row matmuls into PSUM),
  - the linear LIF "current" recurrence via tensor_tensor_scan directly
    from PSUM (segmented by a decay mask: 0 at each t-run start),
  - the nonlinear "voltage" recurrence as 3 DVE ops per timestep,
  - spikes extracted with one batched is_gt per chunk.
The recurrent layer's matmul is inherently per-timestep; everything else is
batched. Output accumulation (fc2) is folded with ts_weights and reduced on
device; host concatenates the 8 per-core [2,4] outputs.

A post-scheduling legalization pass splits multi-semaphore sync waits onto
injected NOPs (this walrus build allows only one wait per instruction).
"""

import numpy as np
import concourse.bass as bass
import concourse.mybir as mybir
from concourse.tile import TileContext
from concourse.bass_utils import run_bass_kernel_spmd

f32 = mybir.dt.float32
Alu = mybir.AluOpType

B, C1, C2, C3, T, FC = 32, 64, 128, 256, 100, 128
NCORES = 8
BL = B // NCORES        # 4 local batch
TC = 10                 # timestep chunk
NCH = T // TC
CD, VD, VTH = 0.5, 0.75, 0.5

_CACHE: dict = {}


def _legalize_sync_waits(nc, max_w=1):
    """Split >max_w sync waits per instruction onto same-engine NOPs."""
    for f in nc.m.functions:
        for blk in f.blocks:
            out = []
            for inst in blk.instructions:
                si = getattr(inst, "sync_info", None)
                ow = list(si.on_wait) if si is not None and si.on_wait else []
                if len(ow) > max_w:
                    extra, keep = ow[:-max_w], ow[-max_w:]
                    for k, w in enumerate(extra):
                        nop = mybir.InstNoOp(name=f"{inst.name}-w{k}")
                        nop.engine = inst.engine
                        nop.sync_info = mybir.SyncInfo(on_wait=[w], on_update=[])
                        out.append(nop)
                    inst.sync_info = mybir.SyncInfo(
                        on_wait=keep, on_update=list(si.on_update))
                out.append(inst)
            blk.instructions[:] = out


def _build_nc(debug=False, repeat=1, ablate=()):
    nc = bass.Bass("TRN2")

    def din(name, shape):
        return nc.dram_tensor(name, shape, f32, kind="ExternalInput")

    rhs1_d = din("rhs1", [9, 2 * 2 * 64 * T])
    w1T_d = din("w1T", [9, 64])
    b1_d = din("b1dup", [1, 128])
    w2T_d = din("w2T", [64, 9 * 128])
    b2_d = din("b2row", [1, 128])
    w3T_d = din("w3T", [128, 9 * 2 * 128])
    b3_d = din("b3row", [1, 256])
    tcw_d = din("tcwT", [128, 3 * 2 * 2 * 128])
    tcbs_d = din("tcbsum", [1, 256])
    tcb01_d = din("tcb01", [128, 2])
    tcb0_d = din("tcb0", [128, 2])
    recw_d = din("recwT", [128, 2 * 2 * 128])
    recb_d = din("recbrow", [1, 256])
    f1w_d = din("fc1wT", [128, 2 * 128])
    f1b_d = din("fc1brow", [1, 128])
    f2w_d = din("fc2wT", [128, 2])
    id_d = din("ident", [128, 128])
    dec_d = din("decay", [128, 1440])
    mrep_d = din("mrep", [128, 4 * TC])
    d0fc_d = din("d0fc", [128, 4 * TC])
    halfm_d = din("halfm", [128, 4])
    wt_d = din("wtrep", [128, 4 * T])
    out_d = nc.dram_tensor("out", [2, 4], f32, kind="ExternalOutput")
    dbg = {}
    if debug:
        for nm, w in [("s1", 1280), ("s2", 1440), ("s3", 80), ("s4", 80),
                      ("s5", 80), ("s6", 40), ("cur1", 1280), ("vol1", 1280),
                      ("cur2", 1440), ("cur4", 80), ("cur6", 40)]:
            dbg[nm] = nc.dram_tensor("dbg_" + nm, [128, w * NCH], f32,
                                     kind="ExternalOutput")

    with TileContext(nc) as tc:
        with (
            tc.tile_pool(name="const", bufs=1) as cp,
            tc.tile_pool(name="big", bufs=2) as bp,
            tc.tile_pool(name="small", bufs=2) as sp,
            tc.tile_pool(name="ktmp", bufs=3) as kp_pool,
            tc.tile_pool(name="psconv", bufs=2, space="PSUM") as pconv,
            tc.tile_pool(name="pstail", bufs=2, space="PSUM") as ptail,
            tc.tile_pool(name="psrec", bufs=1, space="PSUM") as prec,
            tc.tile_pool(name="psfc", bufs=2, space="PSUM") as pfc,
        ):
            # ---- resident constants ----
            w1T = cp.tile([9, 64], f32)
            nc.sync.dma_start(w1T, w1T_d[:])
            b1 = cp.tile([1, 128], f32)
            nc.sync.dma_start(b1, b1_d[:])
            w2T = cp.tile([128, 9 * 128], f32)
            nc.sync.dma_start(w2T[0:64, :], w2T_d[:])
            nc.sync.dma_start(w2T[64:128, :], w2T_d[:])
            b2 = cp.tile([1, 128], f32)
            nc.sync.dma_start(b2, b2_d[:])
            w3T = cp.tile([128, 9 * 2 * 128], f32)
            nc.sync.dma_start(w3T, w3T_d[:])
            b3 = cp.tile([1, 256], f32)
            nc.sync.dma_start(b3, b3_d[:])
            tcw = cp.tile([128, 12 * 128], f32)
            nc.sync.dma_start(tcw, tcw_d[:])
            tcbs = cp.tile([1, 256], f32)
            nc.sync.dma_start(tcbs, tcbs_d[:])
            tcb01 = cp.tile([128, 2], f32)
            nc.sync.dma_start(tcb01, tcb01_d[:])
            tcb0 = cp.tile([128, 2], f32)
            nc.sync.dma_start(tcb0, tcb0_d[:])
            recw = cp.tile([128, 4 * 128], f32)
            nc.sync.dma_start(recw, recw_d[:])
            recb = cp.tile([1, 256], f32)
            nc.sync.dma_start(recb, recb_d[:])
            f1w = cp.tile([128, 2 * 128], f32)
            nc.sync.dma_start(f1w, f1w_d[:])
            f1b = cp.tile([1, 128], f32)
            nc.sync.dma_start(f1b, f1b_d[:])
            f2w = cp.tile([128, 2], f32)
            nc.sync.dma_start(f2w, f2w_d[:])
            ident = cp.tile([128, 128], f32)
            nc.sync.dma_start(ident, id_d[:])
            decay = cp.tile([128, 1440], f32)
            nc.sync.dma_start(decay, dec_d[:])
            mrep = cp.tile([128, 4, TC], f32)
            nc.sync.dma_start(mrep, mrep_d[:].rearrange("p (b t) -> p b t", t=TC))
            d0fc = cp.tile([128, 4 * TC], f32)
            nc.sync.dma_start(d0fc, d0fc_d[:])
            halfm = cp.tile([128, 4], f32)
            nc.sync.dma_start(halfm, halfm_d[:])
            wtrep = cp.tile([128, 4, T], f32)
            nc.sync.dma_start(wtrep, wt_d[:].rearrange("p (b t) -> p b t", t=T))

            ones = cp.tile([1, 512], f32)
            nc.vector.memset(ones, 1.0)
            zl1 = cp.tile([128, 2, 64], f32)
            nc.vector.memset(zl1, 0.0)
            zl2 = cp.tile([128, 4, 36], f32)
            nc.vector.memset(zl2, 0.0)
            zs = cp.tile([128, 2, 4], f32)
            nc.vector.memset(zs, 0.0)
            zf = cp.tile([128, 4], f32)
            nc.vector.memset(zf, 0.0)

            cur5 = cp.tile([128, 2, 4], f32)
            vol5 = cp.tile([128, 2, 4], f32)
            accT = cp.tile([2, 4], f32)

            rhs1v = rhs1_d[:].rearrange(
                "p (bh bl s t) -> p bh bl s t", bh=2, bl=2, s=64)

            def vchain(volc, curc, zero_tile, prev_vol, nseg_dims, kp_name):
                """per-t voltage chain: vol[t]=VD*vol*(vol<=VTH)+cur[t]."""
                if "vchain" in ablate:
                    nc.vector.tensor_copy(out=volc[:], in_=curc[:])
                    return
                for t in range(TC):
                    if t > 0:
                        vprev = volc[(slice(None),) + nseg_dims + (t - 1,)]
                    elif prev_vol is not None:
                        vprev = prev_vol[(slice(None),) + nseg_dims + (TC - 1,)]
                    else:
                        vprev = zero_tile[:]
                    kp = kp_pool.tile(list(zero_tile.shape), f32, tag=kp_name)
                    nc.vector.tensor_scalar(
                        out=kp[:], in0=vprev, scalar1=VTH, scalar2=VD,
                        op0=Alu.is_le, op1=Alu.mult)
                    nc.vector.tensor_tensor(
                        out=kp[:], in0=vprev, in1=kp[:], op=Alu.mult)
                    nc.vector.tensor_tensor(
                        out=volc[(slice(None),) + nseg_dims + (t,)],
                        in0=kp[:],
                        in1=curc[(slice(None),) + nseg_dims + (t,)],
                        op=Alu.add)

            def one_pass():
                prev: dict = {}
                nc.vector.memset(cur5, 0.0)
                nc.vector.memset(vol5, 0.0)
                nc.vector.memset(accT, 0.0)
                for c in range(NCH):
                  t0 = c * TC
                  # ============ conv1 + LIF1 ============
                  rhs1c = bp.tile([9, 2, 2, 64, TC], f32)
                  nc.sync.dma_start(rhs1c, rhs1v[:, :, :, :, t0:t0 + TC])
                  cur1 = bp.tile([128, 2, 64, TC], f32)
                  for bl in range(2):
                      for sh in range(2):
                          ps1 = pconv.tile([128, 32, TC], f32, tag="psconv")
                          nc.tensor.matmul(
                              ps1[:, :, :], b1[:], ones[0:1, 0:32 * TC],
                              start=True, stop=False, skip_group_check=True)
                          for bh in range(2):
                              nc.tensor.matmul(
                                  ps1[64 * bh:64 * bh + 64, :, :], w1T[:],
                                  rhs1c[:, bh, bl, 32 * sh:32 * sh + 32, :],
                                  start=False, stop=(bh == 1),
                                  tile_position=(0, 64 * bh),
                                  skip_group_check=True)
                          if c > 0:
                              nc.vector.scalar_tensor_tensor(
                                  ps1[:, :, 0:1],
                                  prev["cur1"][:, bl, 32 * sh:32 * sh + 32,
                                               TC - 1:TC],
                                  CD, ps1[:, :, 0:1], Alu.mult, Alu.add)
                          nc.vector.tensor_tensor_scan(
                              cur1[:, bl, 32 * sh:32 * sh + 32, :].rearrange(
                                  "p s t -> p (s t)"),
                              decay[:, 0:32 * TC],
                              ps1.rearrange("p s t -> p (s t)"),
                              0.0, Alu.mult, Alu.add)
                  vol1 = bp.tile([128, 2, 64, TC], f32)
                  vchain(vol1, cur1, zl1, prev.get("vol1"), (slice(None),) * 2,
                         "kp1")
                  s1 = bp.tile([128, 2, 64, TC], f32)
                  nc.vector.tensor_scalar(
                      out=s1[:], in0=vol1[:], scalar1=VTH, scalar2=None,
                      op0=Alu.is_gt)

                  # ============ conv2 + LIF2 ============
                  s1v = s1.rearrange("p bl (y x) t -> p bl y x t", y=8)
                  cur2 = bp.tile([128, 4, 36, TC], f32)
                  for bh in range(2):
                      for bl in range(2):
                          bidx = 2 * bh + bl
                          ps2 = pconv.tile([128, 6, 6, TC], f32, tag="psconv")
                          nc.tensor.matmul(
                              ps2[:, :, :, :], b2[:], ones[0:1, 0:360],
                              start=True, stop=False)
                          for tap in range(9):
                              dy, dx = tap // 3, tap % 3
                              nc.tensor.matmul(
                                  ps2[:, :, :, :],
                                  w2T[64 * bh:64 * bh + 64,
                                      tap * 128:(tap + 1) * 128],
                                  s1v[64 * bh:64 * bh + 64, bl,
                                      dy:dy + 6, dx:dx + 6, :],
                                  start=False, stop=(tap == (0 if 'conv2taps' in ablate else 8)))
                          ps2f = ps2.rearrange("p y x t -> p (y x) t")
                          if c > 0:
                              nc.vector.scalar_tensor_tensor(
                                  ps2f[:, :, 0:1],
                                  prev["cur2"][:, bidx, :, TC - 1:TC],
                                  CD, ps2f[:, :, 0:1], Alu.mult, Alu.add)
                          nc.vector.tensor_tensor_scan(
                              cur2[:, bidx, :, :].rearrange("p s t -> p (s t)"),
                              decay[:, 0:360],
                              ps2.rearrange("p y x t -> p (y x t)"),
                              0.0, Alu.mult, Alu.add)
                  vol2 = bp.tile([128, 4, 36, TC], f32)
                  vchain(vol2, cur2, zl2, prev.get("vol2"), (slice(None),) * 2,
                         "kp2")
                  s2 = bp.tile([128, 4, 36, TC], f32)
                  nc.vector.tensor_scalar(
                      out=s2[:], in0=vol2[:], scalar1=VTH, scalar2=None,
                      op0=Alu.is_gt)

                  # ============ avgpool (x0.25 folded into w3) ============
                  s2v = s2.rearrange("p b (q r x) t -> p b q r x t", q=3, r=2)
                  pool1 = bp.tile([128, 4, 3, 6, TC], f32)
                  nc.vector.tensor_tensor(
                      out=pool1[:], in0=s2v[:, :, :, 0, :, :],
                      in1=s2v[:, :, :, 1, :, :], op=Alu.add)
                  p1v = pool1.rearrange("p b q (xq xr) t -> p b q xq xr t", xq=3)
                  p2c = bp.tile([128, 4, 3, 3, TC], f32)
                  nc.vector.tensor_tensor(
                      out=p2c[:], in0=p1v[:, :, :, :, 0, :],
                      in1=p1v[:, :, :, :, 1, :], op=Alu.add)

                  # ============ conv3 + LIF3 ============
                  ps3 = ptail.tile([128, 2, 4, TC], f32, tag="pstail")
                  for h in range(2):
                      nc.tensor.matmul(
                          ps3[:, h, :, :], b3[0:1, h * 128:(h + 1) * 128],
                          ones[0:1, 0:4 * TC], start=True, stop=False)
                      for tap in range(9):
                          dy, dx = tap // 3, tap % 3
                          nc.tensor.matmul(
                              ps3[:, h, :, :],
                              w3T[:, (tap * 2 + h) * 128:(tap * 2 + h + 1) * 128],
                              p2c[:, :, dy, dx, :],
                              start=False, stop=(tap == (0 if 'conv2taps' in ablate else 8)))
                  if c > 0:
                      nc.vector.scalar_tensor_tensor(
                          ps3[:, :, :, 0:1], prev["cur3"][:, :, :, TC - 1:TC],
                          CD, ps3[:, :, :, 0:1], Alu.mult, Alu.add)
                  cur3 = sp.tile([128, 2, 4, TC], f32)
                  nc.vector.tensor_tensor_scan(
                      cur3.rearrange("p h b t -> p (h b t)"),
                      decay[:, 0:80],
                      ps3.rearrange("p h b t -> p (h b t)"),
                      0.0, Alu.mult, Alu.add)
                  vol3 = sp.tile([128, 2, 4, TC], f32)
                  vchain(vol3, cur3, zs, prev.get("vol3"), (slice(None),) * 2,
                         "kp3")
                  s3 = sp.tile([128, 2, 4, TC], f32)
                  nc.vector.tensor_scalar(
                      out=s3[:], in0=vol3[:], scalar1=VTH, scalar2=None,
                      op0=Alu.is_gt)

                  # ============ temporal conv + LIF4 ============
                  # psp_tc[t] = sum_k Wk @ s3[t-2+k] + sum_k tc_b[k] (fixups at
                  # global t in {0,1})
                  ps4 = ptail.tile([128, 2, 4, TC], f32, tag="pstail")
                  for ho in range(2):
                      nc.tensor.matmul(
                          ps4[:, ho, :, :], tcbs[0:1, ho * 128:(ho + 1) * 128],
                          ones[0:1, 0:4 * TC], start=True, stop=False)
                      mms = []
                      for k in range(3):
                          sh_off = k - 2  # source t offset
                          for hi in range(2):
                              lhs = tcw[:, (k * 4 + hi * 2 + ho) * 128:
                                        (k * 4 + hi * 2 + ho + 1) * 128]
                              lo = max(0, -sh_off)
                              mms.append((ps4[:, ho, :, lo:TC], lhs,
                                          s3[:, hi, :, 0:TC - lo]))
                              if lo > 0 and c > 0:
                                  mms.append((ps4[:, ho, :, 0:lo], lhs,
                                              prev["s3"][:, hi, :, TC - lo:TC]))
                      for i, (o, l, r) in enumerate(mms):
                          nc.tensor.matmul(o, l, r, start=False,
                                           stop=(i == len(mms) - 1))
                  if c == 0:
                      for h in range(2):
                          nc.vector.tensor_scalar(
                              out=ps4[:, h, :, 0:1], in0=ps4[:, h, :, 0:1],
                              scalar1=tcb01[:, h:h + 1], scalar2=None,
                              op0=Alu.subtract)
                          nc.vector.tensor_scalar(
                              out=ps4[:, h, :, 1:2], in0=ps4[:, h, :, 1:2],
                              scalar1=tcb0[:, h:h + 1], scalar2=None,
                              op0=Alu.subtract)
                  else:
                      nc.vector.scalar_tensor_tensor(
                          ps4[:, :, :, 0:1], prev["cur4"][:, :, :, TC - 1:TC],
                          CD, ps4[:, :, :, 0:1], Alu.mult, Alu.add)
                  cur4 = sp.tile([128, 2, 4, TC], f32)
                  nc.vector.tensor_tensor_scan(
                      cur4.rearrange("p h b t -> p (h b t)"),
                      decay[:, 0:80],
                      ps4.rearrange("p h b t -> p (h b t)"),
                      0.0, Alu.mult, Alu.add)
                  vol4 = sp.tile([128, 2, 4, TC], f32)
                  vchain(vol4, cur4, zs, prev.get("vol4"), (slice(None),) * 2,
                         "kp4")
                  s4 = sp.tile([128, 2, 4, TC], f32)
                  nc.vector.tensor_scalar(
                      out=s4[:], in0=vol4[:], scalar1=VTH, scalar2=None,
                      op0=Alu.is_gt)

                  # ============ recurrent layer (per-t) ============
                  s5c = sp.tile([128, 2, 4, TC], f32)
                  for t in range(TC):
                      tg = t0 + t
                      psR = prec.tile([128, 2, 4], f32, tag="psR")
                      for ho in range(2):
                          started = False
                          if tg > 0:
                              for hi in range(2):
                                  if t > 0:
                                      s5src = s5c[:, hi, :, t - 1]
                                  else:
                                      s5src = prev["s5"][:, hi, :, TC - 1]
                                  nc.tensor.matmul(
                                      psR[:, ho, :],
                                      recw[:, (hi * 2 + ho) * 128:
                                           (hi * 2 + ho + 1) * 128],
                                      s5src, start=(not started), stop=False)
                                  started = True
                          nc.tensor.matmul(
                              psR[:, ho, :], ident[:], s4[:, ho, :, t],
                              start=(not started), stop=False)
                          nc.tensor.matmul(
                              psR[:, ho, :], recb[0:1, ho * 128:(ho + 1) * 128],
                              ones[0:1, 0:4], start=False, stop=True)
                      nc.vector.scalar_tensor_tensor(
                          cur5[:], cur5[:], CD, psR[:], Alu.mult, Alu.add)
                      kp5 = kp_pool.tile([128, 2, 4], f32, tag="kp5")
                      nc.vector.tensor_scalar(
                          out=kp5[:], in0=vol5[:], scalar1=VTH, scalar2=VD,
                          op0=Alu.is_le, op1=Alu.mult)
                      nc.vector.tensor_tensor(
                          out=kp5[:], in0=vol5[:], in1=kp5[:], op=Alu.mult)
                      nc.vector.tensor_tensor(
                          out=vol5[:], in0=kp5[:], in1=cur5[:], op=Alu.add)
                      nc.vector.tensor_scalar(
                          out=s5c[:, :, :, t], in0=vol5[:], scalar1=VTH,
                          scalar2=None, op0=Alu.is_gt)

                  # ============ fc1 (dropout folded) + LIF6 ============
                  ps6 = pfc.tile([128, 4, TC], f32, tag="psfc")
                  for hi in range(2):
                      nc.tensor.matmul(
                          ps6[:, :, :], f1w[:, hi * 128:(hi + 1) * 128],
                          s5c[:, hi, :, :], start=(hi == 0), stop=False)
                  nc.tensor.matmul(
                      ps6[:, :, :], f1b[:], ones[0:1, 0:4 * TC],
                      start=False, stop=True)
                  d1 = sp.tile([128, 4, TC], f32)
                  nc.vector.tensor_tensor(
                      out=d1[:], in0=ps6[:], in1=mrep[:], op=Alu.mult)
                  if c > 0:
                      tmp4 = kp_pool.tile([128, 4], f32, tag="tmp4")
                      nc.vector.tensor_tensor(
                          out=tmp4[:], in0=prev["cur6"][:, :, TC - 1],
                          in1=halfm[:], op=Alu.mult)
                      nc.vector.tensor_tensor(
                          out=d1[:, :, 0], in0=d1[:, :, 0], in1=tmp4[:],
                          op=Alu.add)
                  cur6 = sp.tile([128, 4, TC], f32)
                  nc.vector.tensor_tensor_scan(
                      cur6.rearrange("p b t -> p (b t)"), d0fc[:],
                      d1.rearrange("p b t -> p (b t)"), 0.0, Alu.mult, Alu.add)
                  vol6 = sp.tile([128, 4, TC], f32)
                  vchain(vol6, cur6, zf, prev.get("vol6"), (slice(None),),
                         "kp6")
                  s6 = sp.tile([128, 4, TC], f32)
                  nc.vector.tensor_scalar(
                      out=s6[:], in0=vol6[:], scalar1=VTH, scalar2=None,
                      op0=Alu.is_gt)

                  # ============ fc2 weighted accumulate ============
                  s6w = sp.tile([128, 4, TC], f32)
                  nc.vector.tensor_tensor(
                      out=s6w[:], in0=s6[:], in1=wtrep[:, :, t0:t0 + TC],
                      op=Alu.mult)
                  psY = pfc.tile([2, 4, TC], f32, tag="psfc")
                  nc.tensor.matmul(
                      psY[:, :, :], f2w[:],
                      s6w.rearrange("p b t -> p (b t)"),
                      start=True, stop=True)
                  red = kp_pool.tile([2, 4], f32, tag="red")
                  nc.vector.tensor_reduce(
                      out=red[:], in_=psY[:, :, :], axis=mybir.AxisListType.X,
                      op=Alu.add)
                  nc.vector.tensor_tensor(
                      out=accT[:], in0=accT[:], in1=red[:], op=Alu.add)

                  if debug:
                      for nm, tl in [("s1", s1), ("s2", s2), ("s3", s3),
                                     ("s4", s4), ("s5", s5c), ("s6", s6),
                                     ("cur1", cur1), ("vol1", vol1),
                                     ("cur2", cur2), ("cur4", cur4),
                                     ("cur6", cur6)]:
                          w = int(np.prod(tl.shape[1:]))
                          nc.sync.dma_start(
                              dbg[nm][:, c * w:(c + 1) * w],
                              tl.rearrange("p ... -> p (...)"))

                  prev = {"cur1": cur1, "vol1": vol1, "cur2": cur2,
                          "vol2": vol2, "cur3": cur3, "vol3": vol3, "s3": s3,
                          "cur4": cur4, "vol4": vol4, "s5": s5c, "cur6": cur6,
                          "vol6": vol6}


            for _rep in range(repeat):
                one_pass()

            nc.sync.dma_start(out_d[:], accT[:])

    _legalize_sync_waits(nc)
    return nc


def _prep_inputs(inputs):
    """Host-side: shard + layout aux arrays per core."""
    x = np.asarray(inputs["input_data"], np.float32)       # [B,1,10,10,T]
    conv1_w = np.asarray(inputs["conv1_w"], np.float32)
    conv1_b = np.asarray(inputs["conv1_b"], np.float32)
    conv2_w = np.asarray(inputs["conv2_w"], np.float32)
    conv2_b = np.asarray(inputs["conv2_b"], np.float32)
    conv3_w = np.asarray(inputs["conv3_w"], np.float32)
    conv3_b = np.asarray(inputs["conv3_b"], np.float32)
    tc_w = np.asarray(inputs["tc_w"], np.float32)
    tc_b = np.asarray(inputs["tc_b"], np.float32)
    rec_w = np.asarray(inputs["rec_w"], np.float32)
    rec_b = np.asarray(inputs["rec_b"], np.float32)
    fc1_w = np.asarray(inputs["fc1_w"], np.float32)
    fc1_b = np.asarray(inputs["fc1_b"], np.float32)
    fc2_w = np.asarray(inputs["fc2_w"], np.float32)
    ts_w = np.asarray(inputs["ts_weights"], np.float32)[:, 0]  # [T]
    mask = np.asarray(inputs["mask_fc"], np.float32)       # [B,FC]

    com = {}
    com["w1T"] = np.ascontiguousarray(conv1_w.reshape(C1, 9).T)
    com["b1dup"] = np.concatenate([conv1_b, conv1_b])[None]
    com["w2T"] = np.ascontiguousarray(
        conv2_w.reshape(C2, C1, 9).transpose(1, 2, 0).reshape(C1, 9 * C2))
    com["b2row"] = conv2_b[None]
    com["w3T"] = np.ascontiguousarray(
        (conv3_w.reshape(C3, C2, 9) * 0.25).transpose(1, 2, 0)
        .reshape(C2, 9, 2, 128).reshape(C2, 9 * 2 * 128))
    com["b3row"] = conv3_b[None]
    tcwT = np.zeros((128, 3, 2, 2, 128), np.float32)
    for k in range(3):
        w = tc_w[k]  # [d_out, c_in] (psp = ins @ tc_w[k] over last axis c)
        for hi in range(2):
            for ho in range(2):
                tcwT[:, k, hi, ho, :] = w[ho * 128:(ho + 1) * 128,
                                          hi * 128:(hi + 1) * 128].T
    com["tcwT"] = tcwT.reshape(128, -1)
    com["tcbsum"] = tc_b.sum(0)[None]
    com["tcb01"] = np.ascontiguousarray((tc_b[0] + tc_b[1]).reshape(2, 128).T)
    com["tcb0"] = np.ascontiguousarray(tc_b[0].reshape(2, 128).T)
    recwT = np.zeros((128, 2, 2, 128), np.float32)
    for hi in range(2):
        for ho in range(2):
            recwT[:, hi, ho, :] = rec_w[ho * 128:(ho + 1) * 128,
                                        hi * 128:(hi + 1) * 128].T
    com["recwT"] = recwT.reshape(128, -1)
    com["recbrow"] = rec_b[None]
    f1wT = np.zeros((128, 2, 128), np.float32)
    for hi in range(2):
        f1wT[:, hi, :] = fc1_w[:, hi * 128:(hi + 1) * 128].T
    com["fc1wT"] = f1wT.reshape(128, -1)
    com["fc1brow"] = fc1_b[None]
    com["fc2wT"] = np.ascontiguousarray(fc2_w.T)
    com["ident"] = np.eye(128, dtype=np.float32)
    dec = np.full((128, 1440), CD, np.float32)
    dec[:, 0::TC] = 0.0
    com["decay"] = dec
    com["wtrep"] = np.broadcast_to(
        ts_w[None, None, :], (128, 4, T)).reshape(128, 4 * T).copy()

    in_maps = []
    for core in range(NCORES):
        b0 = core * BL
        rhs1 = np.zeros((9, 2, 2, 64, T), np.float32)
        for dy in range(3):
            for dx in range(3):
                tap = dy * 3 + dx
                blk = x[b0:b0 + BL, 0, dy:dy + 8, dx:dx + 8, :]  # [4,8,8,T]
                rhs1[tap] = blk.reshape(2, 2, 64, T)
        m = mask[b0:b0 + BL].T  # [FC, 4]
        mrep = np.repeat(m[:, :, None], TC, axis=2)
        d0 = 0.5 * mrep.copy()
        d0[:, :, 0] = 0.0
        im = dict(com)
        im["rhs1"] = rhs1.reshape(9, -1)
        im["mrep"] = mrep.reshape(128, -1)
        im["d0fc"] = d0.reshape(128, -1)
        im["halfm"] = (0.5 * m).copy()
        in_maps.append({k: np.ascontiguousarray(v, np.float32)
                        for k, v in im.items()})
    return in_maps


def kernel(**inputs) -> np.ndarray:
    if "nc" not in _CACHE:
        _CACHE["nc"] = _build_nc()
    nc = _CACHE["nc"]
    in_maps = _prep_inputs(inputs)
    res = run_bass_kernel_spmd(nc, in_maps, core_ids=list(range(NCORES)))
    outs = [r["out"] for r in res.results]  # each [2, 4]
    return np.concatenate([o.T for o in outs], axis=0).astype(np.float32)

